# revision 13
# baseline (speedup 1.0000x reference)
"""Trainium2 Bass kernel for nn_Attention_48541720379807.

Multi-head attention (N=8 heads, H=128) with per-head K/Q projections,
softmax over projected keys, attention applied to projected keys, head
concat, and an output Linear.  B=8, L=2048, E=1024.

Sharding: pure data parallel - batch element b -> NeuronCore b.  No
collectives.

Fully fused single-pass structure (v1):
  - kT/qT src tiles (8MB bf16) loaded once and kept SBUF-resident; no
    DRAM scratch roundtrips at all (baseline spilled kxT/qxT/on).
  - per head n: scores/exp/AV run with head n+1's projection matmuls
    (kxT/qxT chains) interleaved as PE filler between score pairs, so
    the in-order PE queue never stalls on ACT exp latency.
  - denominator: 4-level pairwise DVE fold tree (16 exp tiles -> 1),
    then a single ones128 matmul per q-block (baseline used 4).
  - engine placement: ACT = exp + projection evictions; DVE = folds,
    kx_nat copies, recip, normalize mul, bias add; gpsimd = w/pw DMA;
    PE = all matmuls + transposes.
  - phase C (output projection) reads head outputs straight from SBUF.

PSUM (16KB/partition, 8 banks): psS scores 2x(128,1024)f32 = 4 banks,
psA projection chains 2x(128,512)f32 = 2, psO attn-out 1, psD
denom/transpose 1.
"""

import math
import os
from collections import deque

import numpy as np

B, L, E, N, H = 8, 2048, 1024, 8, 128
NCORES = 8
QBLK = 512          # q block width in phase B
KCH = L // 128      # 16 k chunks
ECH = E // 128      # 8 e chunks
SCALE = 1.0 / math.sqrt(H)

MODE = "bf16"

_CACHE = {}
_last_in_maps = None


def _build(mode):
    from concourse import bacc
    import concourse.mybir as mybir
    from concourse.tile import TileContext
    from concourse.masks import make_identity

    assert mode == "bf16", "fused kernel supports bf16 only"
    f32 = mybir.dt.float32
    mdt = mybir.dt.bfloat16

    nc = bacc.Bacc("TRN2", target_bir_lowering=False, debug=False,
                   num_devices=NCORES)

    kT_d = nc.dram_tensor("kT", [E, L], mdt, kind="ExternalInput")
    qT_d = nc.dram_tensor("qT", [E, L], mdt, kind="ExternalInput")
    # w in (p, n, ec, h) layout: per-head slice = 2KB contiguous lines
    wk_d = nc.dram_tensor("wk", [128, N * ECH * H], mdt,
                          kind="ExternalInput")
    wq_d = nc.dram_tensor("wq", [128, N * ECH * H], mdt,
                          kind="ExternalInput")
    pwT_d = nc.dram_tensor("pwT", [N * H, E], mdt, kind="ExternalInput")
    pb_d = nc.dram_tensor("pb", [1, E], f32, kind="ExternalInput")
    y_d = nc.dram_tensor("y", [L, E], f32, kind="ExternalOutput")

    with TileContext(nc) as tc:
        with (
            tc.tile_pool(name="const", bufs=1) as const,
            tc.tile_pool(name="srcp", bufs=1) as srcp,     # 32x(128,1024) kT/qT
            tc.tile_pool(name="wsl", bufs=2) as wsl,       # w tiles, 2 tags
            tc.tile_pool(name="kxth", bufs=2) as kxth,     # per-head kxT
            tc.tile_pool(name="qxh", bufs=2) as qxh,       # per-head qxT
            tc.tile_pool(name="kxn", bufs=1) as kxn,       # per-head kx_nat
            tc.tile_pool(name="onh", bufs=1) as onh,       # 8 resident on tiles
            tc.tile_pool(name="expp", bufs=9) as expp,     # exp pairs
            tc.tile_pool(name="fldp", bufs=3) as fldp,     # fold levels
            tc.tile_pool(name="pwp", bufs=1) as pwp,       # 8 pw tiles
            tc.tile_pool(name="small", bufs=2) as small,   # d_rc, y_sb
            tc.tile_pool(name="psS", bufs=2, space="PSUM") as psS,
            tc.tile_pool(name="psA", bufs=int(os.environ.get("KPSA", "1")),
                         space="PSUM") as psA,
            tc.tile_pool(name="psO", bufs=int(os.environ.get("KPSO", "2")),
                         space="PSUM") as psO,
            tc.tile_pool(name="psD", bufs=1, space="PSUM") as psD,
        ):
            ident_f = const.tile([128, 128], f32)
            make_identity(nc, ident_f)
            ident = const.tile([128, 128], mdt)
            nc.vector.tensor_copy(ident[:], ident_f[:])
            ones128_f = const.tile([128, 128], f32)
            nc.any.memset(ones128_f[:], 1.0)
            ones128 = const.tile([128, 128], mdt)
            nc.vector.tensor_copy(ones128[:], ones128_f[:])
            # ---------------- DMA issue helpers ----------------
            def load_w(w_d, n, tag):
                wt = wsl.tile([128, ECH * H], mdt, tag=tag, name=f"w_{tag}{n}")
                nc.gpsimd.dma_start(
                    out=wt[:],
                    in_=w_d[:, n * ECH * H:(n + 1) * ECH * H])
                return wt

            wk_t = {0: load_w(wk_d, 0, "wk"), 1: load_w(wk_d, 1, "wk")}
            wq_t = {0: load_w(wq_d, 0, "wq"), 1: load_w(wq_d, 1, "wq")}

            # resident src tiles: src[tensor][lh][ec] = (128,1024)
            src = {"k": [[None] * ECH for _ in range(2)],
                   "q": [[None] * ECH for _ in range(2)]}
            for tn, src_d in (("k", kT_d), ("q", qT_d)):
                for lh in range(2):
                    for ec in range(ECH):
                        st = srcp.tile([128, 1024], mdt,
                                       tag=f"s{tn}{lh}{ec}", name=f"src_{tn}{lh}{ec}")
                        eng = nc.sync if ec % 2 == 0 else nc.scalar
                        eng.dma_start(
                            out=st[:],
                            in_=src_d[ec * 128:(ec + 1) * 128,
                                      lh * 1024:(lh + 1) * 1024])
                        src[tn][lh][ec] = st

            pb_sb = const.tile([1, E], f32)
            nc.gpsimd.dma_start(out=pb_sb[:], in_=pb_d[:])
            pb_bc = const.tile([128, E], f32)
            nc.gpsimd.partition_broadcast(pb_bc[:], pb_sb[:])

            def load_pw(c):
                pwt = pwp.tile([128, E], mdt, tag=f"pw{c}", name=f"pw{c}")
                nc.gpsimd.dma_start(out=pwt[:],
                                    in_=pwT_d[c * 128:(c + 1) * 128, :])
                return pwt

            # ---------------- A chains (projections) ----------------
            def new_proj_tiles(n):
                kt_ = kxth.tile([128, L], mdt, tag="kx", name=f"kxT{n}")
                qt_ = qxh.tile([128, L], mdt, tag="qx", name=f"qxT{n}")
                return kt_, qt_

            def chain_units(wt, stiles, dst, c):
                # one 512-wide output block of dst = sum_ec w[ec].T @ src
                lh, lb = c // 2, c % 2
                state = {}

                def unit(ec):
                    if ec == 0:
                        state["ps"] = psA.tile([128, 512], f32, tag="a",
                                               name="psa")
                    nc.tensor.matmul(
                        state["ps"][:],
                        wt[:, ec * H:(ec + 1) * H],
                        stiles[lh][ec][:, lb * 512:(lb + 1) * 512],
                        start=(ec == 0), stop=(ec == ECH - 1))
                    if ec == ECH - 1:
                        nc.scalar.copy(dst[:, c * 512:(c + 1) * 512],
                                       state["ps"][:])
                return [lambda ec=ec: unit(ec) for ec in range(ECH)]

            def proj_units(n, kt_, qt_):
                units = []
                for c in range(4):
                    units += chain_units(wk_t[n], src["k"], kt_, c)
                for c in range(4):
                    units += chain_units(wq_t[n], src["q"], qt_, c)
                fmode = os.environ.get("KFILL", "fine")
                if fmode == "atomic":
                    # one closure per 8-matmul chain, so chains are never
                    # split by other PE instructions
                    units = [
                        (lambda us=units[i * 8:(i + 1) * 8]:
                         [u() for u in us])
                        for i in range(8)
                    ]
                elif fmode == "none":
                    for u in units:
                        u()
                    units = []
                return units

            # ---------------- transposes -> kx_nat ----------------
            def transpose_units(kxT, kx_nat):
                def unit(g):
                    pt = psD.tile([128, 512], mdt, tag="d", name="pst")
                    for j in range(4):
                        kc = 4 * g + j
                        nc.tensor.transpose(
                            pt[:, j * 128:(j + 1) * 128],
                            kxT[:, kc * 128:(kc + 1) * 128], ident[:])
                    nc.vector.tensor_copy(
                        kx_nat[:, g * 512:(g + 1) * 512], pt[:])
                return [lambda g=g: unit(g) for g in range(4)]

            # ---------------- denominator flush ----------------
            pending = []    # (f4, ps_o, on_tile, qs)

            def flush_denoms(keep=0):
                while len(pending) > keep:
                    f4, ps_o_t, on_t, qs_ = pending.pop(0)
                    ps_d = psD.tile([128, QBLK], f32, tag="d", name="psd")
                    nc.tensor.matmul(ps_d[:], ones128[:], f4[:],
                                     start=True, stop=True)
                    d_rc = small.tile([128, QBLK], f32, tag="drc",
                                      name="drc")
                    nc.vector.reciprocal_approx_fast(d_rc[:], ps_d[:])
                    nc.vector.tensor_mul(on_t[:, qs_], ps_o_t[:], d_rc[:])

            # ---------------- per-head emission ----------------
            fillers = deque()

            def fill(k):
                for _ in range(k):
                    if fillers:
                        fillers.popleft()()

            def run_head(n, kxT, qxT, kx_nat, on_t):
                for qb in range(4):
                    qs = slice(qb * QBLK, (qb + 1) * QBLK)
                    pairs, f1s, f2s, f3s = [], [], [], []
                    for p in range(KCH // 2):
                        if p >= 2:
                            fill(3 if qb == 0 else 2)
                        ps_s = psS.tile([128, 2 * QBLK], f32, tag="s",
                                        name="pss")
                        for j in range(2):
                            kt = 2 * p + j
                            nc.tensor.matmul(
                                ps_s[:, j * QBLK:(j + 1) * QBLK],
                                kxT[:, kt * 128:(kt + 1) * 128],
                                qxT[:, qs], start=True, stop=True)
                        et = expp.tile([128, 2 * QBLK], mdt, tag="e",
                                       name="expt")
                        nc.scalar.activation(
                            et[:], ps_s[:],
                            mybir.ActivationFunctionType.Exp, scale=SCALE)
                        pairs.append(et)
                        f1 = fldp.tile([128, QBLK], mdt, tag="f1",
                                       name="f1")
                        nc.vector.tensor_add(f1[:], et[:, :QBLK],
                                             et[:, QBLK:])
                        f1s.append(f1)
                        if p % 2 == 1:
                            f2 = fldp.tile([128, QBLK], mdt, tag="f2",
                                           bufs=2, name="f2")
                            nc.vector.tensor_add(f2[:], f1s[-2][:],
                                                 f1s[-1][:])
                            f2s.append(f2)
                        if p in (3, 7):
                            f3 = fldp.tile([128, QBLK], mdt, tag="f3",
                                           bufs=2, name="f3")
                            nc.vector.tensor_add(f3[:], f2s[-2][:],
                                                 f2s[-1][:])
                            f3s.append(f3)
                    f4 = fldp.tile([128, QBLK], mdt, tag="f4", bufs=2,
                                   name="f4")
                    nc.vector.tensor_add(f4[:], f3s[0][:], f3s[1][:])
                    fill(6 if qb == 0 else 4)
                    flush_denoms(keep=1)
                    ps_o = psO.tile([128, QBLK], f32, tag="o", name="pso")
                    for kc in range(KCH):
                        nc.tensor.matmul(
                            ps_o[:],
                            kx_nat[:, kc * H:(kc + 1) * H],
                            pairs[kc // 2][:, (kc % 2) * QBLK:
                                           (kc % 2 + 1) * QBLK],
                            start=(kc == 0), stop=(kc == KCH - 1))
                    pending.append((f4, ps_o, on_t, qs))

            # ---------------- prologue: head 0 k-chains + first q-chain ----
            # B0 can start once kxT(0) and qxT(0)[:, :512] exist; the other
            # three q-chains of head 0 become B0 fillers, so PE starts ~20us
            # earlier (src DMA is HBM-paced).
            kxt0, qxt0 = new_proj_tiles(0)
            units0 = proj_units(0, kxt0, qxt0)
            with nc.named_scope("A0"):
                for u in units0[:40]:
                    u()
            q_tail0 = units0[40:]

            on_tiles = []
            pw_tiles = []
            cur = (kxt0, qxt0)
            for n in range(N):
                with nc.named_scope(f"B{n}"):
                    kxT, qxT = cur
                    kx_nat = kxn.tile([128, KCH * H], mdt, tag="kxn",
                                      name=f"kxn{n}")
                    on_t = onh.tile([128, L], mdt, tag=f"on{n}",
                                    name=f"on{n}")
                    on_tiles.append(on_t)

                    # DMA issue for future heads
                    if n + 2 < N:
                        wk_t[n + 2] = load_w(wk_d, n + 2, "wk")
                        wq_t[n + 2] = load_w(wq_d, n + 2, "wq")
                    if n == 1:
                        for c in range(4):
                            pw_tiles.append(load_pw(c))
                    elif n == 2:
                        for c in range(4, 8):
                            pw_tiles.append(load_pw(c))

                    for u in reversed(transpose_units(kxT, kx_nat)):
                        fillers.appendleft(u)
                    if n == 0:
                        fillers.extend(q_tail0)
                    if n + 1 < N:
                        nxt = new_proj_tiles(n + 1)
                        fillers.extend(proj_units(n + 1, *nxt))
                    run_head(n, kxT, qxT, kx_nat, on_t)
                    if n + 1 < N:
                        cur = nxt

            # ---------------- phase C ----------------
            with nc.named_scope("C"):
                flush_denoms(keep=0)
                for qt in range(L // 128):
                    ps_y = psS.tile([128, 1024], f32, tag="s", name="psy")
                    for eb in range(2):
                        for c in range(N):
                            nc.tensor.matmul(
                                ps_y[:, eb * 512:(eb + 1) * 512],
                                on_tiles[c][:, qt * 128:(qt + 1) * 128],
                                pw_tiles[c][:, eb * 512:(eb + 1) * 512],
                                start=(c == 0), stop=(c == N - 1))
                    y_sb = small.tile([128, E], f32, tag="ysb", name="ysb")
                    nc.vector.tensor_add(y_sb[:], ps_y[:], pb_bc[:])
                    eng = (nc.sync, nc.scalar, nc.gpsimd)[qt % 3]
                    eng.dma_start(out=y_d[qt * 128:(qt + 1) * 128, :],
                                  in_=y_sb[:])

    nc.compile()
    return nc


def _get_program(mode=MODE):
    if mode not in _CACHE:
        _CACHE[mode] = _build(mode)
    return _CACHE[mode]


def kernel(k, q, w_kx, w_qx, proj_w, proj_b, mode=MODE):
    from concourse.bass_utils import run_bass_kernel_spmd
    import ml_dtypes

    k = np.asarray(k, dtype=np.float32)
    q = np.asarray(q, dtype=np.float32)
    w_kx = np.asarray(w_kx, dtype=np.float32)
    w_qx = np.asarray(w_qx, dtype=np.float32)
    proj_w = np.asarray(proj_w, dtype=np.float32)
    proj_b = np.asarray(proj_b, dtype=np.float32)

    rnd = lambda x: np.asarray(x, dtype=np.float32).astype(ml_dtypes.bfloat16)
    # (p, n, ec, h) layout: per-head slice has 2KB contiguous lines
    wk = rnd(np.ascontiguousarray(
        w_kx.reshape(N, ECH, 128, H).transpose(2, 0, 1, 3).reshape(
            128, N * ECH * H)))
    wq = rnd(np.ascontiguousarray(
        w_qx.reshape(N, ECH, 128, H).transpose(2, 0, 1, 3).reshape(
            128, N * ECH * H)))
    pwT = rnd(proj_w.T)
    pb = np.ascontiguousarray(proj_b.reshape(1, E), dtype=np.float32)

    in_maps = []
    for b in range(NCORES):
        in_maps.append({
            "kT": rnd(k[b].T),
            "qT": rnd(q[b].T),
            "wk": wk,
            "wq": wq,
            "pwT": pwT,
            "pb": pb,
        })

    global _last_in_maps
    _last_in_maps = in_maps
    nc = _get_program(mode)
    res = run_bass_kernel_spmd(nc, in_maps, list(range(NCORES)))
    out = np.stack([res.results[b]["y"] for b in range(NCORES)], axis=0)
    return out.astype(np.float32)


# revision 15
# speedup vs baseline: 1.0111x; 1.0111x over previous
"""Trainium2 Bass kernel for nn_Attention_48541720379807.

Multi-head attention (N=8 heads, H=128) with per-head K/Q projections,
softmax over projected keys, attention applied to projected keys, head
concat, and an output Linear.  B=8, L=2048, E=1024.

Sharding: pure data parallel - batch element b -> NeuronCore b.  No
collectives.

Fully fused single-pass structure (v1):
  - kT/qT src tiles (8MB bf16) loaded once and kept SBUF-resident; no
    DRAM scratch roundtrips at all (baseline spilled kxT/qxT/on).
  - per head n: scores/exp/AV run with head n+1's projection matmuls
    (kxT/qxT chains) interleaved as PE filler between score pairs, so
    the in-order PE queue never stalls on ACT exp latency.
  - denominator: 4-level pairwise DVE fold tree (16 exp tiles -> 1),
    then a single ones128 matmul per q-block (baseline used 4).
  - engine placement: ACT = exp + projection evictions; DVE = folds,
    kx_nat copies, recip, normalize mul, bias add; gpsimd = w/pw DMA;
    PE = all matmuls + transposes.
  - phase C (output projection) reads head outputs straight from SBUF.

PSUM (16KB/partition, 8 banks): psS scores 2x(128,1024)f32 = 4 banks,
psA projection chains 2x(128,512)f32 = 2, psO attn-out 1, psD
denom/transpose 1.
"""

import math
import os
from collections import deque

import numpy as np

B, L, E, N, H = 8, 2048, 1024, 8, 128
NCORES = 8
QBLK = 512          # q block width in phase B
KCH = L // 128      # 16 k chunks
ECH = E // 128      # 8 e chunks
SCALE = 1.0 / math.sqrt(H)

MODE = "bf16"

_CACHE = {}
_last_in_maps = None


def _build(mode):
    from concourse import bacc
    import concourse.mybir as mybir
    from concourse.tile import TileContext
    from concourse.masks import make_identity

    assert mode == "bf16", "fused kernel supports bf16 only"
    f32 = mybir.dt.float32
    mdt = mybir.dt.bfloat16

    nc = bacc.Bacc("TRN2", target_bir_lowering=False, debug=False,
                   num_devices=NCORES)

    kT_d = nc.dram_tensor("kT", [E, L], mdt, kind="ExternalInput")
    qT_d = nc.dram_tensor("qT", [E, L], mdt, kind="ExternalInput")
    # w in (p, n, ec, h) layout: per-head slice = 2KB contiguous lines
    wk_d = nc.dram_tensor("wk", [128, N * ECH * H], mdt,
                          kind="ExternalInput")
    wq_d = nc.dram_tensor("wq", [128, N * ECH * H], mdt,
                          kind="ExternalInput")
    pwT_d = nc.dram_tensor("pwT", [N * H, E], mdt, kind="ExternalInput")
    pb_d = nc.dram_tensor("pb", [1, E], f32, kind="ExternalInput")
    y_d = nc.dram_tensor("y", [L, E], f32, kind="ExternalOutput")

    with TileContext(nc) as tc:
        with (
            tc.tile_pool(name="const", bufs=1) as const,
            tc.tile_pool(name="srcp", bufs=1) as srcp,     # 32x(128,1024) kT/qT
            tc.tile_pool(name="wsl", bufs=2) as wsl,       # w tiles, 2 tags
            tc.tile_pool(name="kxth", bufs=2) as kxth,     # per-head kxT
            tc.tile_pool(name="qxh", bufs=2) as qxh,       # per-head qxT
            tc.tile_pool(name="kxn", bufs=1) as kxn,       # per-head kx_nat
            tc.tile_pool(name="onh", bufs=1) as onh,       # 8 resident on tiles
            tc.tile_pool(name="expp", bufs=9) as expp,     # exp pairs
            tc.tile_pool(name="fldp", bufs=3) as fldp,     # fold levels
            tc.tile_pool(name="pwp", bufs=1) as pwp,       # 8 pw tiles
            tc.tile_pool(name="small", bufs=2) as small,   # d_rc, y_sb
            tc.tile_pool(name="psS", bufs=2, space="PSUM") as psS,
            tc.tile_pool(name="psA", bufs=int(os.environ.get("KPSA", "1")),
                         space="PSUM") as psA,
            tc.tile_pool(name="psO", bufs=int(os.environ.get("KPSO", "2")),
                         space="PSUM") as psO,
            tc.tile_pool(name="psD", bufs=1, space="PSUM") as psD,
        ):
            ident_f = const.tile([128, 128], f32)
            make_identity(nc, ident_f)
            ident = const.tile([128, 128], mdt)
            nc.vector.tensor_copy(ident[:], ident_f[:])
            ones128_f = const.tile([128, 128], f32)
            nc.any.memset(ones128_f[:], 1.0)
            ones128 = const.tile([128, 128], mdt)
            nc.vector.tensor_copy(ones128[:], ones128_f[:])
            # ---------------- DMA issue helpers ----------------
            def load_w(w_d, n, tag):
                wt = wsl.tile([128, ECH * H], mdt, tag=tag, name=f"w_{tag}{n}")
                nc.gpsimd.dma_start(
                    out=wt[:],
                    in_=w_d[:, n * ECH * H:(n + 1) * ECH * H])
                return wt

            wk_t = {0: load_w(wk_d, 0, "wk"), 1: load_w(wk_d, 1, "wk")}
            wq_t = {0: load_w(wq_d, 0, "wq"), 1: load_w(wq_d, 1, "wq")}

            # resident src tiles: src[tensor][ec] = (128, L) full rows, 4KB
            # lines.  k strictly before q on BOTH rings so kxT completes
            # first; scalar ring stays DMA-free (a dma_start blocks the
            # issuing engine's queue until the transfer drains, which would
            # stall exp/evict work behind it).
            src = {"k": [None] * ECH, "q": [None] * ECH}
            for tn, src_d in (("k", kT_d), ("q", qT_d)):
                for ec in range(ECH):
                    st = srcp.tile([128, L], mdt,
                                   tag=f"s{tn}{ec}", name=f"src_{tn}{ec}")
                    eng = nc.sync if ec % 2 == 0 else nc.gpsimd
                    eng.dma_start(out=st[:],
                                  in_=src_d[ec * 128:(ec + 1) * 128, :])
                    src[tn][ec] = st

            pb_sb = const.tile([1, E], f32)
            nc.gpsimd.dma_start(out=pb_sb[:], in_=pb_d[:])
            pb_bc = const.tile([128, E], f32)
            nc.gpsimd.partition_broadcast(pb_bc[:], pb_sb[:])

            def load_pw(c):
                pwt = pwp.tile([128, E], mdt, tag=f"pw{c}", name=f"pw{c}")
                nc.gpsimd.dma_start(out=pwt[:],
                                    in_=pwT_d[c * 128:(c + 1) * 128, :])
                return pwt

            # ---------------- A chains (projections) ----------------
            def new_proj_tiles(n):
                kt_ = kxth.tile([128, L], mdt, tag="kx", name=f"kxT{n}")
                qt_ = qxh.tile([128, L], mdt, tag="qx", name=f"qxT{n}")
                return kt_, qt_

            def chain_units(wt, stiles, dst, c):
                # one 512-wide output block of dst = sum_ec w[ec].T @ src
                state = {}

                def unit(ec):
                    if ec == 0:
                        state["ps"] = psA.tile([128, 512], f32, tag="a",
                                               name="psa")
                    nc.tensor.matmul(
                        state["ps"][:],
                        wt[:, ec * H:(ec + 1) * H],
                        stiles[ec][:, c * 512:(c + 1) * 512],
                        start=(ec == 0), stop=(ec == ECH - 1))
                    if ec == ECH - 1:
                        nc.scalar.copy(dst[:, c * 512:(c + 1) * 512],
                                       state["ps"][:])
                return [lambda ec=ec: unit(ec) for ec in range(ECH)]

            def proj_units(n, kt_, qt_):
                units = []
                for c in range(4):
                    units += chain_units(wk_t[n], src["k"], kt_, c)
                for c in range(4):
                    units += chain_units(wq_t[n], src["q"], qt_, c)
                fmode = os.environ.get("KFILL", "fine")
                if fmode == "atomic":
                    # one closure per 8-matmul chain, so chains are never
                    # split by other PE instructions
                    units = [
                        (lambda us=units[i * 8:(i + 1) * 8]:
                         [u() for u in us])
                        for i in range(8)
                    ]
                elif fmode == "none":
                    for u in units:
                        u()
                    units = []
                return units

            # ---------------- transposes -> kx_nat ----------------
            def transpose_units(kxT, kx_nat):
                def unit(g):
                    pt = psD.tile([128, 512], mdt, tag="d", name="pst")
                    for j in range(4):
                        kc = 4 * g + j
                        nc.tensor.transpose(
                            pt[:, j * 128:(j + 1) * 128],
                            kxT[:, kc * 128:(kc + 1) * 128], ident[:])
                    nc.vector.tensor_copy(
                        kx_nat[:, g * 512:(g + 1) * 512], pt[:])
                return [lambda g=g: unit(g) for g in range(4)]

            # ---------------- denominator flush ----------------
            pending = []    # (f4, ps_o, on_tile, qs)

            def flush_denoms(keep=0):
                while len(pending) > keep:
                    f4, ps_o_t, on_t, qs_ = pending.pop(0)
                    ps_d = psD.tile([128, QBLK], f32, tag="d", name="psd")
                    nc.tensor.matmul(ps_d[:], ones128[:], f4[:],
                                     start=True, stop=True)
                    d_rc = small.tile([128, QBLK], f32, tag="drc",
                                      name="drc")
                    nc.vector.reciprocal_approx_fast(d_rc[:], ps_d[:])
                    nc.vector.tensor_mul(on_t[:, qs_], ps_o_t[:], d_rc[:])

            # ---------------- per-head emission ----------------
            fillers = deque()

            def fill(k):
                for _ in range(k):
                    if fillers:
                        fillers.popleft()()

            def run_head(n, kxT, qxT, kx_nat, on_t):
                for qb in range(4):
                    qs = slice(qb * QBLK, (qb + 1) * QBLK)
                    pairs, f1s, f2s, f3s = [], [], [], []
                    for p in range(KCH // 2):
                        if p >= 2:
                            fill(3 if qb == 0 else 2)
                        ps_s = psS.tile([128, 2 * QBLK], f32, tag="s",
                                        name="pss")
                        for j in range(2):
                            kt = 2 * p + j
                            nc.tensor.matmul(
                                ps_s[:, j * QBLK:(j + 1) * QBLK],
                                kxT[:, kt * 128:(kt + 1) * 128],
                                qxT[:, qs], start=True, stop=True)
                        et = expp.tile([128, 2 * QBLK], mdt, tag="e",
                                       name="expt")
                        nc.scalar.activation(
                            et[:], ps_s[:],
                            mybir.ActivationFunctionType.Exp, scale=SCALE)
                        pairs.append(et)
                        f1 = fldp.tile([128, QBLK], mdt, tag="f1",
                                       name="f1")
                        nc.vector.tensor_add(f1[:], et[:, :QBLK],
                                             et[:, QBLK:])
                        f1s.append(f1)
                        if p % 2 == 1:
                            f2 = fldp.tile([128, QBLK], mdt, tag="f2",
                                           bufs=2, name="f2")
                            nc.vector.tensor_add(f2[:], f1s[-2][:],
                                                 f1s[-1][:])
                            f2s.append(f2)
                        if p in (3, 7):
                            f3 = fldp.tile([128, QBLK], mdt, tag="f3",
                                           bufs=2, name="f3")
                            nc.vector.tensor_add(f3[:], f2s[-2][:],
                                                 f2s[-1][:])
                            f3s.append(f3)
                    f4 = fldp.tile([128, QBLK], mdt, tag="f4", bufs=2,
                                   name="f4")
                    nc.vector.tensor_add(f4[:], f3s[0][:], f3s[1][:])
                    fill(6 if qb == 0 else 4)
                    flush_denoms(keep=1)
                    ps_o = psO.tile([128, QBLK], f32, tag="o", name="pso")
                    for kc in range(KCH):
                        nc.tensor.matmul(
                            ps_o[:],
                            kx_nat[:, kc * H:(kc + 1) * H],
                            pairs[kc // 2][:, (kc % 2) * QBLK:
                                           (kc % 2 + 1) * QBLK],
                            start=(kc == 0), stop=(kc == KCH - 1))
                    pending.append((f4, ps_o, on_t, qs))

            # ---------------- prologue: head 0 k-chains + first q-chain ----
            # B0 can start once kxT(0) and qxT(0)[:, :512] exist; the other
            # three q-chains of head 0 become B0 fillers, so PE starts ~20us
            # earlier (src DMA is HBM-paced).
            kxt0, qxt0 = new_proj_tiles(0)
            units0 = proj_units(0, kxt0, qxt0)
            with nc.named_scope("A0"):
                for u in units0[:40]:
                    u()
            q_tail0 = units0[40:]

            on_tiles = []
            pw_tiles = []
            cur = (kxt0, qxt0)
            for n in range(N):
                with nc.named_scope(f"B{n}"):
                    kxT, qxT = cur
                    kx_nat = kxn.tile([128, KCH * H], mdt, tag="kxn",
                                      name=f"kxn{n}")
                    on_t = onh.tile([128, L], mdt, tag=f"on{n}",
                                    name=f"on{n}")
                    on_tiles.append(on_t)

                    # DMA issue for future heads
                    if n + 2 < N:
                        wk_t[n + 2] = load_w(wk_d, n + 2, "wk")
                        wq_t[n + 2] = load_w(wq_d, n + 2, "wq")
                    if n == 1:
                        for c in range(4):
                            pw_tiles.append(load_pw(c))
                    elif n == 2:
                        for c in range(4, 8):
                            pw_tiles.append(load_pw(c))

                    for u in reversed(transpose_units(kxT, kx_nat)):
                        fillers.appendleft(u)
                    if n == 0:
                        fillers.extend(q_tail0)
                    if n + 1 < N:
                        nxt = new_proj_tiles(n + 1)
                        fillers.extend(proj_units(n + 1, *nxt))
                    run_head(n, kxT, qxT, kx_nat, on_t)
                    if n + 1 < N:
                        cur = nxt

            # ---------------- phase C ----------------
            with nc.named_scope("C"):
                flush_denoms(keep=0)
                for qt in range(L // 128):
                    ps_y = psS.tile([128, 1024], f32, tag="s", name="psy")
                    for eb in range(2):
                        for c in range(N):
                            nc.tensor.matmul(
                                ps_y[:, eb * 512:(eb + 1) * 512],
                                on_tiles[c][:, qt * 128:(qt + 1) * 128],
                                pw_tiles[c][:, eb * 512:(eb + 1) * 512],
                                start=(c == 0), stop=(c == N - 1))
                    y_sb = small.tile([128, E], f32, tag="ysb", name="ysb")
                    nc.vector.tensor_add(y_sb[:], ps_y[:], pb_bc[:])
                    eng = (nc.sync, nc.scalar, nc.gpsimd)[qt % 3]
                    eng.dma_start(out=y_d[qt * 128:(qt + 1) * 128, :],
                                  in_=y_sb[:])

    nc.compile()
    return nc


def _get_program(mode=MODE):
    if mode not in _CACHE:
        _CACHE[mode] = _build(mode)
    return _CACHE[mode]


def kernel(k, q, w_kx, w_qx, proj_w, proj_b, mode=MODE):
    from concourse.bass_utils import run_bass_kernel_spmd
    import ml_dtypes

    k = np.asarray(k, dtype=np.float32)
    q = np.asarray(q, dtype=np.float32)
    w_kx = np.asarray(w_kx, dtype=np.float32)
    w_qx = np.asarray(w_qx, dtype=np.float32)
    proj_w = np.asarray(proj_w, dtype=np.float32)
    proj_b = np.asarray(proj_b, dtype=np.float32)

    rnd = lambda x: np.asarray(x, dtype=np.float32).astype(ml_dtypes.bfloat16)
    # (p, n, ec, h) layout: per-head slice has 2KB contiguous lines
    wk = rnd(np.ascontiguousarray(
        w_kx.reshape(N, ECH, 128, H).transpose(2, 0, 1, 3).reshape(
            128, N * ECH * H)))
    wq = rnd(np.ascontiguousarray(
        w_qx.reshape(N, ECH, 128, H).transpose(2, 0, 1, 3).reshape(
            128, N * ECH * H)))
    pwT = rnd(proj_w.T)
    pb = np.ascontiguousarray(proj_b.reshape(1, E), dtype=np.float32)

    in_maps = []
    for b in range(NCORES):
        in_maps.append({
            "kT": rnd(k[b].T),
            "qT": rnd(q[b].T),
            "wk": wk,
            "wq": wq,
            "pwT": pwT,
            "pb": pb,
        })

    global _last_in_maps
    _last_in_maps = in_maps
    nc = _get_program(mode)
    res = run_bass_kernel_spmd(nc, in_maps, list(range(NCORES)))
    out = np.stack([res.results[b]["y"] for b in range(NCORES)], axis=0)
    return out.astype(np.float32)


# revision 16
# speedup vs baseline: 1.0326x; 1.0212x over previous
"""Trainium2 Bass kernel for nn_Attention_48541720379807.

Multi-head attention (N=8 heads, H=128) with per-head K/Q projections,
softmax over projected keys, attention applied to projected keys, head
concat, and an output Linear.  B=8, L=2048, E=1024.

Sharding: pure data parallel - batch element b -> NeuronCore b.  No
collectives.

Fully fused single-pass structure (v1):
  - kT/qT src tiles (8MB bf16) loaded once and kept SBUF-resident; no
    DRAM scratch roundtrips at all (baseline spilled kxT/qxT/on).
  - per head n: scores/exp/AV run with head n+1's projection matmuls
    (kxT/qxT chains) interleaved as PE filler between score pairs, so
    the in-order PE queue never stalls on ACT exp latency.
  - denominator: 4-level pairwise DVE fold tree (16 exp tiles -> 1),
    then a single ones128 matmul per q-block (baseline used 4).
  - engine placement: ACT = exp + projection evictions; DVE = folds,
    kx_nat copies, recip, normalize mul, bias add; gpsimd = w/pw DMA;
    PE = all matmuls + transposes.
  - phase C (output projection) reads head outputs straight from SBUF.

PSUM (16KB/partition, 8 banks): psS scores 2x(128,1024)f32 = 4 banks,
psA projection chains 2x(128,512)f32 = 2, psO attn-out 1, psD
denom/transpose 1.
"""

import math
import os
from collections import deque

import numpy as np

B, L, E, N, H = 8, 2048, 1024, 8, 128
NCORES = 8
QBLK = 512          # q block width in phase B
KCH = L // 128      # 16 k chunks
ECH = E // 128      # 8 e chunks
SCALE = 1.0 / math.sqrt(H)

MODE = "bf16"

_CACHE = {}
_last_in_maps = None


def _build(mode):
    from concourse import bacc
    import concourse.mybir as mybir
    from concourse.tile import TileContext
    from concourse.masks import make_identity

    assert mode == "bf16", "fused kernel supports bf16 only"
    f32 = mybir.dt.float32
    mdt = mybir.dt.bfloat16

    nc = bacc.Bacc("TRN2", target_bir_lowering=False, debug=False,
                   num_devices=NCORES)

    kT_d = nc.dram_tensor("kT", [E, L], mdt, kind="ExternalInput")
    qT_d = nc.dram_tensor("qT", [E, L], mdt, kind="ExternalInput")
    # w in (p, n, ec, h) layout: per-head slice = 2KB contiguous lines
    wk_d = nc.dram_tensor("wk", [128, N * ECH * H], mdt,
                          kind="ExternalInput")
    wq_d = nc.dram_tensor("wq", [128, N * ECH * H], mdt,
                          kind="ExternalInput")
    pwT_d = nc.dram_tensor("pwT", [N * H, E], mdt, kind="ExternalInput")
    pb_d = nc.dram_tensor("pb", [1, E], f32, kind="ExternalInput")
    y_d = nc.dram_tensor("y", [L, E], f32, kind="ExternalOutput")

    with TileContext(nc) as tc:
        with (
            tc.tile_pool(name="const", bufs=1) as const,
            tc.tile_pool(name="srcp", bufs=1) as srcp,     # 32x(128,1024) kT/qT
            tc.tile_pool(name="wsl", bufs=2) as wsl,       # w tiles, 2 tags
            tc.tile_pool(name="kxth", bufs=2) as kxth,     # per-head kxT
            tc.tile_pool(name="qxh", bufs=2) as qxh,       # per-head qxT
            tc.tile_pool(name="kxn", bufs=1) as kxn,       # per-head kx_nat
            tc.tile_pool(name="onh", bufs=1) as onh,       # 8 resident on tiles
            tc.tile_pool(name="expp", bufs=9) as expp,     # exp pairs
            tc.tile_pool(name="fldp", bufs=3) as fldp,     # fold levels
            tc.tile_pool(name="pwp", bufs=1) as pwp,       # 8 pw tiles
            tc.tile_pool(name="small", bufs=2) as small,   # d_rc, y_sb
            tc.tile_pool(name="psS", bufs=2, space="PSUM") as psS,
            tc.tile_pool(name="psA", bufs=int(os.environ.get("KPSA", "1")),
                         space="PSUM") as psA,
            tc.tile_pool(name="psO", bufs=int(os.environ.get("KPSO", "2")),
                         space="PSUM") as psO,
            tc.tile_pool(name="psD", bufs=1, space="PSUM") as psD,
        ):
            ident_f = const.tile([128, 128], f32)
            make_identity(nc, ident_f)
            ident = const.tile([128, 128], mdt)
            nc.vector.tensor_copy(ident[:], ident_f[:])
            ones128_f = const.tile([128, 128], f32)
            nc.any.memset(ones128_f[:], 1.0)
            ones128 = const.tile([128, 128], mdt)
            nc.vector.tensor_copy(ones128[:], ones128_f[:])
            # ---------------- DMA issue helpers ----------------
            def load_w(w_d, n, tag):
                wt = wsl.tile([128, ECH * H], mdt, tag=tag, name=f"w_{tag}{n}")
                nc.gpsimd.dma_start(
                    out=wt[:],
                    in_=w_d[:, n * ECH * H:(n + 1) * ECH * H])
                return wt

            wk_t = {0: load_w(wk_d, 0, "wk"), 1: load_w(wk_d, 1, "wk")}
            wq_t = {0: load_w(wq_d, 0, "wq"), 1: load_w(wq_d, 1, "wq")}

            # resident src tiles: src[tensor][ec] = (128, L) full rows, 4KB
            # lines.  k strictly before q on BOTH rings so kxT completes
            # first; scalar ring stays DMA-free (a dma_start blocks the
            # issuing engine's queue until the transfer drains, which would
            # stall exp/evict work behind it).
            src = {"k": [None] * ECH, "q": [None] * ECH}
            for tn in ("k", "q"):
                for ec in range(ECH):
                    src[tn][ec] = srcp.tile(
                        [128, L], mdt, tag=f"s{tn}{ec}", name=f"src_{tn}{ec}")
            # issue in half-row units so chain c's subtile dep gates on the
            # earliest transfer that covers it: k-lh0, k-lh1, q-lh0, q-lh1
            for tn, src_d in (("k", kT_d), ("q", qT_d)):
                for lh in range(2):
                    for ec in range(ECH):
                        ls = slice(lh * 1024, (lh + 1) * 1024)
                        eng = nc.sync if ec % 2 == 0 else nc.gpsimd
                        eng.dma_start(
                            out=src[tn][ec][:, ls],
                            in_=src_d[ec * 128:(ec + 1) * 128, ls])

            pb_sb = const.tile([1, E], f32)
            nc.gpsimd.dma_start(out=pb_sb[:], in_=pb_d[:])
            pb_bc = const.tile([128, E], f32)
            nc.gpsimd.partition_broadcast(pb_bc[:], pb_sb[:])

            def load_pw(c):
                pwt = pwp.tile([128, E], mdt, tag=f"pw{c}", name=f"pw{c}")
                nc.gpsimd.dma_start(out=pwt[:],
                                    in_=pwT_d[c * 128:(c + 1) * 128, :])
                return pwt

            # ---------------- A chains (projections) ----------------
            def new_proj_tiles(n):
                kt_ = kxth.tile([128, L], mdt, tag="kx", name=f"kxT{n}")
                qt_ = qxh.tile([128, L], mdt, tag="qx", name=f"qxT{n}")
                return kt_, qt_

            def chain_units(wt, stiles, dst, c):
                # one 512-wide output block of dst = sum_ec w[ec].T @ src
                state = {}

                def unit(ec):
                    if ec == 0:
                        state["ps"] = psA.tile([128, 512], f32, tag="a",
                                               name="psa")
                    nc.tensor.matmul(
                        state["ps"][:],
                        wt[:, ec * H:(ec + 1) * H],
                        stiles[ec][:, c * 512:(c + 1) * 512],
                        start=(ec == 0), stop=(ec == ECH - 1))
                    if ec == ECH - 1:
                        nc.scalar.copy(dst[:, c * 512:(c + 1) * 512],
                                       state["ps"][:])
                return [lambda ec=ec: unit(ec) for ec in range(ECH)]

            def proj_units(n, kt_, qt_):
                units = []
                for c in range(4):
                    units += chain_units(wk_t[n], src["k"], kt_, c)
                for c in range(4):
                    units += chain_units(wq_t[n], src["q"], qt_, c)
                fmode = os.environ.get("KFILL", "fine")
                if fmode == "atomic":
                    # one closure per 8-matmul chain, so chains are never
                    # split by other PE instructions
                    units = [
                        (lambda us=units[i * 8:(i + 1) * 8]:
                         [u() for u in us])
                        for i in range(8)
                    ]
                elif fmode == "none":
                    for u in units:
                        u()
                    units = []
                return units

            # ---------------- transposes -> kx_nat ----------------
            def transpose_units(kxT, kx_nat):
                def unit(g):
                    pt = psD.tile([128, 512], mdt, tag="d", name="pst")
                    for j in range(4):
                        kc = 4 * g + j
                        nc.tensor.transpose(
                            pt[:, j * 128:(j + 1) * 128],
                            kxT[:, kc * 128:(kc + 1) * 128], ident[:])
                    nc.vector.tensor_copy(
                        kx_nat[:, g * 512:(g + 1) * 512], pt[:])
                return [lambda g=g: unit(g) for g in range(4)]

            # ---------------- denominator flush ----------------
            pending = []    # (f4, ps_o, on_tile, qs)

            def flush_denoms(keep=0):
                while len(pending) > keep:
                    f4, ps_o_t, on_t, qs_ = pending.pop(0)
                    ps_d = psD.tile([128, QBLK], f32, tag="d", name="psd")
                    nc.tensor.matmul(ps_d[:], ones128[:], f4[:],
                                     start=True, stop=True)
                    d_rc = small.tile([128, QBLK], f32, tag="drc",
                                      name="drc")
                    nc.vector.reciprocal_approx_fast(d_rc[:], ps_d[:])
                    nc.vector.tensor_mul(on_t[:, qs_], ps_o_t[:], d_rc[:])

            # ---------------- per-head emission ----------------
            fillers = deque()

            def fill(k):
                for _ in range(k):
                    if fillers:
                        fillers.popleft()()

            def run_head(n, kxT, qxT, kx_nat, on_t):
                for qb in range(4):
                    qs = slice(qb * QBLK, (qb + 1) * QBLK)
                    pairs, f1s, f2s, f3s = [], [], [], []
                    for p in range(KCH // 2):
                        if p >= 2:
                            fill(3 if qb == 0 else 2)
                        ps_s = psS.tile([128, 2 * QBLK], f32, tag="s",
                                        name="pss")
                        for j in range(2):
                            kt = 2 * p + j
                            nc.tensor.matmul(
                                ps_s[:, j * QBLK:(j + 1) * QBLK],
                                kxT[:, kt * 128:(kt + 1) * 128],
                                qxT[:, qs], start=True, stop=True)
                        et = expp.tile([128, 2 * QBLK], mdt, tag="e",
                                       name="expt")
                        nc.scalar.activation(
                            et[:], ps_s[:],
                            mybir.ActivationFunctionType.Exp, scale=SCALE)
                        pairs.append(et)
                        f1 = fldp.tile([128, QBLK], mdt, tag="f1",
                                       name="f1")
                        nc.vector.tensor_add(f1[:], et[:, :QBLK],
                                             et[:, QBLK:])
                        f1s.append(f1)
                        if p % 2 == 1:
                            f2 = fldp.tile([128, QBLK], mdt, tag="f2",
                                           bufs=2, name="f2")
                            nc.vector.tensor_add(f2[:], f1s[-2][:],
                                                 f1s[-1][:])
                            f2s.append(f2)
                        if p in (3, 7):
                            f3 = fldp.tile([128, QBLK], mdt, tag="f3",
                                           bufs=2, name="f3")
                            nc.vector.tensor_add(f3[:], f2s[-2][:],
                                                 f2s[-1][:])
                            f3s.append(f3)
                    f4 = fldp.tile([128, QBLK], mdt, tag="f4", bufs=2,
                                   name="f4")
                    nc.vector.tensor_add(f4[:], f3s[0][:], f3s[1][:])
                    fill(6 if qb == 0 else 4)
                    flush_denoms(keep=1)
                    ps_o = psO.tile([128, QBLK], f32, tag="o", name="pso")
                    for kc in range(KCH):
                        nc.tensor.matmul(
                            ps_o[:],
                            kx_nat[:, kc * H:(kc + 1) * H],
                            pairs[kc // 2][:, (kc % 2) * QBLK:
                                           (kc % 2 + 1) * QBLK],
                            start=(kc == 0), stop=(kc == KCH - 1))
                    pending.append((f4, ps_o, on_t, qs))

            # ---------------- prologue: head 0 k-chains + first q-chain ----
            # B0 can start once kxT(0) and qxT(0)[:, :512] exist; the other
            # three q-chains of head 0 become B0 fillers, so PE starts ~20us
            # earlier (src DMA is HBM-paced).
            kxt0, qxt0 = new_proj_tiles(0)
            units0 = proj_units(0, kxt0, qxt0)
            with nc.named_scope("A0"):
                for u in units0[:40]:
                    u()
            q_tail0 = units0[40:]

            on_tiles = []
            pw_tiles = []
            cur = (kxt0, qxt0)
            for n in range(N):
                with nc.named_scope(f"B{n}"):
                    kxT, qxT = cur
                    kx_nat = kxn.tile([128, KCH * H], mdt, tag="kxn",
                                      name=f"kxn{n}")
                    on_t = onh.tile([128, L], mdt, tag=f"on{n}",
                                    name=f"on{n}")
                    on_tiles.append(on_t)

                    # DMA issue for future heads
                    if n + 2 < N:
                        wk_t[n + 2] = load_w(wk_d, n + 2, "wk")
                        wq_t[n + 2] = load_w(wq_d, n + 2, "wq")
                    if n == 1:
                        for c in range(4):
                            pw_tiles.append(load_pw(c))
                    elif n == 2:
                        for c in range(4, 8):
                            pw_tiles.append(load_pw(c))

                    for u in reversed(transpose_units(kxT, kx_nat)):
                        fillers.appendleft(u)
                    if n == 0:
                        fillers.extend(q_tail0)
                    if n + 1 < N:
                        nxt = new_proj_tiles(n + 1)
                        fillers.extend(proj_units(n + 1, *nxt))
                    run_head(n, kxT, qxT, kx_nat, on_t)
                    if n + 1 < N:
                        cur = nxt

            # ---------------- phase C ----------------
            with nc.named_scope("C"):
                flush_denoms(keep=0)
                for qt in range(L // 128):
                    ps_y = psS.tile([128, 1024], f32, tag="s", name="psy")
                    for eb in range(2):
                        for c in range(N):
                            nc.tensor.matmul(
                                ps_y[:, eb * 512:(eb + 1) * 512],
                                on_tiles[c][:, qt * 128:(qt + 1) * 128],
                                pw_tiles[c][:, eb * 512:(eb + 1) * 512],
                                start=(c == 0), stop=(c == N - 1))
                    y_sb = small.tile([128, E], f32, tag="ysb", name="ysb")
                    nc.vector.tensor_add(y_sb[:], ps_y[:], pb_bc[:])
                    eng = (nc.sync, nc.scalar, nc.gpsimd)[qt % 3]
                    eng.dma_start(out=y_d[qt * 128:(qt + 1) * 128, :],
                                  in_=y_sb[:])

    nc.compile()
    return nc


def _get_program(mode=MODE):
    if mode not in _CACHE:
        _CACHE[mode] = _build(mode)
    return _CACHE[mode]


def kernel(k, q, w_kx, w_qx, proj_w, proj_b, mode=MODE):
    from concourse.bass_utils import run_bass_kernel_spmd
    import ml_dtypes

    k = np.asarray(k, dtype=np.float32)
    q = np.asarray(q, dtype=np.float32)
    w_kx = np.asarray(w_kx, dtype=np.float32)
    w_qx = np.asarray(w_qx, dtype=np.float32)
    proj_w = np.asarray(proj_w, dtype=np.float32)
    proj_b = np.asarray(proj_b, dtype=np.float32)

    rnd = lambda x: np.asarray(x, dtype=np.float32).astype(ml_dtypes.bfloat16)
    # (p, n, ec, h) layout: per-head slice has 2KB contiguous lines
    wk = rnd(np.ascontiguousarray(
        w_kx.reshape(N, ECH, 128, H).transpose(2, 0, 1, 3).reshape(
            128, N * ECH * H)))
    wq = rnd(np.ascontiguousarray(
        w_qx.reshape(N, ECH, 128, H).transpose(2, 0, 1, 3).reshape(
            128, N * ECH * H)))
    pwT = rnd(proj_w.T)
    pb = np.ascontiguousarray(proj_b.reshape(1, E), dtype=np.float32)

    in_maps = []
    for b in range(NCORES):
        in_maps.append({
            "kT": rnd(k[b].T),
            "qT": rnd(q[b].T),
            "wk": wk,
            "wq": wq,
            "pwT": pwT,
            "pb": pb,
        })

    global _last_in_maps
    _last_in_maps = in_maps
    nc = _get_program(mode)
    res = run_bass_kernel_spmd(nc, in_maps, list(range(NCORES)))
    out = np.stack([res.results[b]["y"] for b in range(NCORES)], axis=0)
    return out.astype(np.float32)


# revision 18
# speedup vs baseline: 1.0366x; 1.0039x over previous
"""Trainium2 Bass kernel for nn_Attention_48541720379807.

Multi-head attention (N=8 heads, H=128) with per-head K/Q projections,
softmax over projected keys, attention applied to projected keys, head
concat, and an output Linear.  B=8, L=2048, E=1024.

Sharding: pure data parallel - batch element b -> NeuronCore b.  No
collectives.

Fully fused single-pass structure (v1):
  - kT/qT src tiles (8MB bf16) loaded once and kept SBUF-resident; no
    DRAM scratch roundtrips at all (baseline spilled kxT/qxT/on).
  - per head n: scores/exp/AV run with head n+1's projection matmuls
    (kxT/qxT chains) interleaved as PE filler between score pairs, so
    the in-order PE queue never stalls on ACT exp latency.
  - denominator: 4-level pairwise DVE fold tree (16 exp tiles -> 1),
    then a single ones128 matmul per q-block (baseline used 4).
  - engine placement: ACT = exp + projection evictions; DVE = folds,
    kx_nat copies, recip, normalize mul, bias add; gpsimd = w/pw DMA;
    PE = all matmuls + transposes.
  - phase C (output projection) reads head outputs straight from SBUF.

PSUM (16KB/partition, 8 banks): psS scores 2x(128,1024)f32 = 4 banks,
psA projection chains 2x(128,512)f32 = 2, psO attn-out 1, psD
denom/transpose 1.
"""

import math
import os
from collections import deque

import numpy as np

B, L, E, N, H = 8, 2048, 1024, 8, 128
NCORES = 8
QBLK = 512          # q block width in phase B
KCH = L // 128      # 16 k chunks
ECH = E // 128      # 8 e chunks
SCALE = 1.0 / math.sqrt(H)

MODE = "bf16"

_CACHE = {}
_last_in_maps = None


def _build(mode):
    from concourse import bacc
    import concourse.mybir as mybir
    from concourse.tile import TileContext
    from concourse.masks import make_identity

    assert mode == "bf16", "fused kernel supports bf16 only"
    f32 = mybir.dt.float32
    mdt = mybir.dt.bfloat16

    nc = bacc.Bacc("TRN2", target_bir_lowering=False, debug=False,
                   num_devices=NCORES)

    kT_d = nc.dram_tensor("kT", [E, L], mdt, kind="ExternalInput")
    qT_d = nc.dram_tensor("qT", [E, L], mdt, kind="ExternalInput")
    # w in (p, n, ec, h) layout: per-head slice = 2KB contiguous lines
    wk_d = nc.dram_tensor("wk", [128, N * ECH * H], mdt,
                          kind="ExternalInput")
    wq_d = nc.dram_tensor("wq", [128, N * ECH * H], mdt,
                          kind="ExternalInput")
    pwT_d = nc.dram_tensor("pwT", [N * H, E], mdt, kind="ExternalInput")
    pb_d = nc.dram_tensor("pb", [1, E], f32, kind="ExternalInput")
    y_d = nc.dram_tensor("y", [L, E], f32, kind="ExternalOutput")

    with TileContext(nc) as tc:
        with (
            tc.tile_pool(name="const", bufs=1) as const,
            tc.tile_pool(name="srcp", bufs=1) as srcp,     # 32x(128,1024) kT/qT
            tc.tile_pool(name="wsl", bufs=2) as wsl,       # w tiles, 2 tags
            tc.tile_pool(name="kxth", bufs=2) as kxth,     # per-head kxT
            tc.tile_pool(name="qxh", bufs=2) as qxh,       # per-head qxT
            tc.tile_pool(name="kxn", bufs=1) as kxn,       # per-head kx_nat
            tc.tile_pool(name="onh", bufs=1) as onh,       # 8 resident on tiles
            tc.tile_pool(name="expp", bufs=9) as expp,     # exp pairs
            tc.tile_pool(name="fldp", bufs=3) as fldp,     # fold levels
            tc.tile_pool(name="pwp", bufs=1) as pwp,       # 8 pw tiles
            tc.tile_pool(name="small", bufs=2) as small,   # d_rc, y_sb
            tc.tile_pool(name="psS", bufs=2, space="PSUM") as psS,
            tc.tile_pool(name="psA", bufs=int(os.environ.get("KPSA", "1")),
                         space="PSUM") as psA,
            tc.tile_pool(name="psO", bufs=int(os.environ.get("KPSO", "2")),
                         space="PSUM") as psO,
            tc.tile_pool(name="psD", bufs=1, space="PSUM") as psD,
        ):
            ident_f = const.tile([128, 128], f32)
            make_identity(nc, ident_f)
            ident = const.tile([128, 128], mdt)
            nc.vector.tensor_copy(ident[:], ident_f[:])
            ones128_f = const.tile([128, 128], f32)
            nc.any.memset(ones128_f[:], 1.0)
            ones128 = const.tile([128, 128], mdt)
            nc.vector.tensor_copy(ones128[:], ones128_f[:])
            # ---------------- DMA issue helpers ----------------
            def load_w(w_d, n, tag):
                wt = wsl.tile([128, ECH * H], mdt, tag=tag, name=f"w_{tag}{n}")
                nc.gpsimd.dma_start(
                    out=wt[:],
                    in_=w_d[:, n * ECH * H:(n + 1) * ECH * H])
                return wt

            wk_t = {0: load_w(wk_d, 0, "wk"), 1: load_w(wk_d, 1, "wk")}
            wq_t = {0: load_w(wq_d, 0, "wq"), 1: load_w(wq_d, 1, "wq")}

            # resident src tiles: src[tensor][ec] = (128, L) full rows, 4KB
            # lines.  k strictly before q on BOTH rings so kxT completes
            # first; scalar ring stays DMA-free (a dma_start blocks the
            # issuing engine's queue until the transfer drains, which would
            # stall exp/evict work behind it).
            src = {"k": [None] * ECH, "q": [None] * ECH}
            for tn in ("k", "q"):
                for ec in range(ECH):
                    src[tn][ec] = srcp.tile(
                        [128, L], mdt, tag=f"s{tn}{ec}", name=f"src_{tn}{ec}")
            # issue in half-row units so chain c's subtile dep gates on the
            # earliest transfer that covers it: k-lh0, q-lh0, k-lh1, q-lh1
            # (prologue chains k0/k1 run during q-lh0, q-c0 during k-lh1)
            for lh in range(2):
                for tn, src_d in (("k", kT_d), ("q", qT_d)):
                    for ec in range(ECH):
                        ls = slice(lh * 1024, (lh + 1) * 1024)
                        eng = nc.sync if ec % 2 == 0 else nc.gpsimd
                        eng.dma_start(
                            out=src[tn][ec][:, ls],
                            in_=src_d[ec * 128:(ec + 1) * 128, ls])

            pb_sb = const.tile([1, E], f32)
            nc.gpsimd.dma_start(out=pb_sb[:], in_=pb_d[:])
            pb_bc = const.tile([128, E], f32)
            nc.gpsimd.partition_broadcast(pb_bc[:], pb_sb[:])

            def load_pw(c):
                pwt = pwp.tile([128, E], mdt, tag=f"pw{c}", name=f"pw{c}")
                nc.gpsimd.dma_start(out=pwt[:],
                                    in_=pwT_d[c * 128:(c + 1) * 128, :])
                return pwt

            # ---------------- A chains (projections) ----------------
            def new_proj_tiles(n):
                kt_ = kxth.tile([128, L], mdt, tag="kx", name=f"kxT{n}")
                qt_ = qxh.tile([128, L], mdt, tag="qx", name=f"qxT{n}")
                return kt_, qt_

            def chain_units(wt, stiles, dst, c):
                # one 512-wide output block of dst = sum_ec w[ec].T @ src
                state = {}

                def unit(ec):
                    if ec == 0:
                        state["ps"] = psA.tile([128, 512], f32, tag="a",
                                               name="psa")
                    nc.tensor.matmul(
                        state["ps"][:],
                        wt[:, ec * H:(ec + 1) * H],
                        stiles[ec][:, c * 512:(c + 1) * 512],
                        start=(ec == 0), stop=(ec == ECH - 1))
                    if ec == ECH - 1:
                        nc.scalar.copy(dst[:, c * 512:(c + 1) * 512],
                                       state["ps"][:])
                return [lambda ec=ec: unit(ec) for ec in range(ECH)]

            def proj_units(n, kt_, qt_):
                units = []
                for c in range(4):
                    units += chain_units(wk_t[n], src["k"], kt_, c)
                for c in range(4):
                    units += chain_units(wq_t[n], src["q"], qt_, c)
                fmode = os.environ.get("KFILL", "fine")
                if fmode == "atomic":
                    # one closure per 8-matmul chain, so chains are never
                    # split by other PE instructions
                    units = [
                        (lambda us=units[i * 8:(i + 1) * 8]:
                         [u() for u in us])
                        for i in range(8)
                    ]
                elif fmode == "none":
                    for u in units:
                        u()
                    units = []
                return units

            # ---------------- transposes -> kx_nat ----------------
            def transpose_units(kxT, kx_nat):
                def unit(g):
                    pt = psD.tile([128, 512], mdt, tag="d", name="pst")
                    for j in range(4):
                        kc = 4 * g + j
                        nc.tensor.transpose(
                            pt[:, j * 128:(j + 1) * 128],
                            kxT[:, kc * 128:(kc + 1) * 128], ident[:])
                    nc.vector.tensor_copy(
                        kx_nat[:, g * 512:(g + 1) * 512], pt[:])
                return [lambda g=g: unit(g) for g in range(4)]

            # ---------------- denominator flush ----------------
            pending = []    # (f4, ps_o, on_tile, qs)

            def flush_denoms(keep=0):
                while len(pending) > keep:
                    f4, ps_o_t, on_t, qs_ = pending.pop(0)
                    ps_d = psD.tile([128, QBLK], f32, tag="d", name="psd")
                    nc.tensor.matmul(ps_d[:], ones128[:], f4[:],
                                     start=True, stop=True)
                    d_rc = small.tile([128, QBLK], f32, tag="drc",
                                      name="drc")
                    nc.vector.reciprocal_approx_fast(d_rc[:], ps_d[:])
                    nc.vector.tensor_mul(on_t[:, qs_], ps_o_t[:], d_rc[:])

            # ---------------- per-head emission ----------------
            fillers = deque()

            def fill(k):
                for _ in range(k):
                    if fillers:
                        fillers.popleft()()

            def run_head(n, kxT, qxT, kx_nat, on_t):
                for qb in range(4):
                    qs = slice(qb * QBLK, (qb + 1) * QBLK)
                    pairs, f1s, f2s, f3s = [], [], [], []
                    for p in range(KCH // 2):
                        if p >= 2:
                            fill(3 if qb == 0 else 2)
                        ps_s = psS.tile([128, 2 * QBLK], f32, tag="s",
                                        name="pss")
                        for j in range(2):
                            kt = 2 * p + j
                            nc.tensor.matmul(
                                ps_s[:, j * QBLK:(j + 1) * QBLK],
                                kxT[:, kt * 128:(kt + 1) * 128],
                                qxT[:, qs], start=True, stop=True)
                        et = expp.tile([128, 2 * QBLK], mdt, tag="e",
                                       name="expt")
                        nc.scalar.activation(
                            et[:], ps_s[:],
                            mybir.ActivationFunctionType.Exp, scale=SCALE)
                        pairs.append(et)
                        f1 = fldp.tile([128, QBLK], mdt, tag="f1",
                                       name="f1")
                        nc.vector.tensor_add(f1[:], et[:, :QBLK],
                                             et[:, QBLK:])
                        f1s.append(f1)
                        if p % 2 == 1:
                            f2 = fldp.tile([128, QBLK], mdt, tag="f2",
                                           bufs=2, name="f2")
                            nc.vector.tensor_add(f2[:], f1s[-2][:],
                                                 f1s[-1][:])
                            f2s.append(f2)
                        if p in (3, 7):
                            f3 = fldp.tile([128, QBLK], mdt, tag="f3",
                                           bufs=2, name="f3")
                            nc.vector.tensor_add(f3[:], f2s[-2][:],
                                                 f2s[-1][:])
                            f3s.append(f3)
                    f4 = fldp.tile([128, QBLK], mdt, tag="f4", bufs=2,
                                   name="f4")
                    nc.vector.tensor_add(f4[:], f3s[0][:], f3s[1][:])
                    fill(6 if qb == 0 else 4)
                    flush_denoms(keep=1)
                    ps_o = psO.tile([128, QBLK], f32, tag="o", name="pso")
                    for kc in range(KCH):
                        nc.tensor.matmul(
                            ps_o[:],
                            kx_nat[:, kc * H:(kc + 1) * H],
                            pairs[kc // 2][:, (kc % 2) * QBLK:
                                           (kc % 2 + 1) * QBLK],
                            start=(kc == 0), stop=(kc == KCH - 1))
                    pending.append((f4, ps_o, on_t, qs))

            # ---------------- prologue: head 0 k-chains + first q-chain ----
            # B0 can start once kxT(0) and qxT(0)[:, :512] exist; the other
            # three q-chains of head 0 become B0 fillers, so PE starts ~20us
            # earlier (src DMA is HBM-paced).
            kxt0, qxt0 = new_proj_tiles(0)
            units0 = proj_units(0, kxt0, qxt0)
            with nc.named_scope("A0"):
                # k0, k1 (k-lh0), q-c0 (q-lh0, overlaps k-lh1 DMA), k2, k3
                for u in (units0[0:16] + units0[32:40] + units0[16:32]):
                    u()
            q_tail0 = units0[40:]

            on_tiles = []
            pw_tiles = []
            cur = (kxt0, qxt0)
            for n in range(N):
                with nc.named_scope(f"B{n}"):
                    kxT, qxT = cur
                    kx_nat = kxn.tile([128, KCH * H], mdt, tag="kxn",
                                      name=f"kxn{n}")
                    on_t = onh.tile([128, L], mdt, tag=f"on{n}",
                                    name=f"on{n}")
                    on_tiles.append(on_t)

                    # DMA issue for future heads
                    if n + 2 < N:
                        wk_t[n + 2] = load_w(wk_d, n + 2, "wk")
                        wq_t[n + 2] = load_w(wq_d, n + 2, "wq")
                    if n == 1:
                        for c in range(4):
                            pw_tiles.append(load_pw(c))
                    elif n == 2:
                        for c in range(4, 8):
                            pw_tiles.append(load_pw(c))

                    for u in reversed(transpose_units(kxT, kx_nat)):
                        fillers.appendleft(u)
                    if n == 0:
                        fillers.extend(q_tail0)
                    if n + 1 < N:
                        nxt = new_proj_tiles(n + 1)
                        fillers.extend(proj_units(n + 1, *nxt))
                    run_head(n, kxT, qxT, kx_nat, on_t)
                    if n + 1 < N:
                        cur = nxt

            # ---------------- phase C ----------------
            with nc.named_scope("C"):
                flush_denoms(keep=0)
                for qt in range(L // 128):
                    ps_y = psS.tile([128, 1024], f32, tag="s", name="psy")
                    for eb in range(2):
                        for c in range(N):
                            nc.tensor.matmul(
                                ps_y[:, eb * 512:(eb + 1) * 512],
                                on_tiles[c][:, qt * 128:(qt + 1) * 128],
                                pw_tiles[c][:, eb * 512:(eb + 1) * 512],
                                start=(c == 0), stop=(c == N - 1))
                    y_sb = small.tile([128, E], f32, tag="ysb", name="ysb")
                    nc.vector.tensor_add(y_sb[:], ps_y[:], pb_bc[:])
                    eng = (nc.sync, nc.scalar, nc.gpsimd)[qt % 3]
                    eng.dma_start(out=y_d[qt * 128:(qt + 1) * 128, :],
                                  in_=y_sb[:])

    nc.compile()
    return nc


def _get_program(mode=MODE):
    if mode not in _CACHE:
        _CACHE[mode] = _build(mode)
    return _CACHE[mode]


def kernel(k, q, w_kx, w_qx, proj_w, proj_b, mode=MODE):
    from concourse.bass_utils import run_bass_kernel_spmd
    import ml_dtypes

    k = np.asarray(k, dtype=np.float32)
    q = np.asarray(q, dtype=np.float32)
    w_kx = np.asarray(w_kx, dtype=np.float32)
    w_qx = np.asarray(w_qx, dtype=np.float32)
    proj_w = np.asarray(proj_w, dtype=np.float32)
    proj_b = np.asarray(proj_b, dtype=np.float32)

    rnd = lambda x: np.asarray(x, dtype=np.float32).astype(ml_dtypes.bfloat16)
    # (p, n, ec, h) layout: per-head slice has 2KB contiguous lines
    wk = rnd(np.ascontiguousarray(
        w_kx.reshape(N, ECH, 128, H).transpose(2, 0, 1, 3).reshape(
            128, N * ECH * H)))
    wq = rnd(np.ascontiguousarray(
        w_qx.reshape(N, ECH, 128, H).transpose(2, 0, 1, 3).reshape(
            128, N * ECH * H)))
    pwT = rnd(proj_w.T)
    pb = np.ascontiguousarray(proj_b.reshape(1, E), dtype=np.float32)

    in_maps = []
    for b in range(NCORES):
        in_maps.append({
            "kT": rnd(k[b].T),
            "qT": rnd(q[b].T),
            "wk": wk,
            "wq": wq,
            "pwT": pwT,
            "pb": pb,
        })

    global _last_in_maps
    _last_in_maps = in_maps
    nc = _get_program(mode)
    res = run_bass_kernel_spmd(nc, in_maps, list(range(NCORES)))
    out = np.stack([res.results[b]["y"] for b in range(NCORES)], axis=0)
    return out.astype(np.float32)
